# revision 1
# baseline (speedup 1.0000x reference)
"""Sliding-window gated attention on 8 TRN2 NeuronCores.

Sharding: data/sequence parallel, no collectives. 2 batches x 4096 tokens
= 8192 tokens -> 8 shards of 1024 owned tokens (core c: batch c//4,
segment c%4). Each shard also receives a 256-token halo of x on the left
(the sliding window W=256 only ever reaches one block back), so every
core computes its outputs fully locally. For segment-0 cores the halo is
dummy data that the attention mask zeroes out.

Per-core layout is feature-major ("transposed"): xT [1024 dim, 1280 tok].
  rs      = 1/||x_t||           (ones-vector matmul over squared chunks)
  xhatT   = xT * rs             (row-broadcast via gpsimd partition_broadcast)
  qT,kT   = W^T @ xhatT         [feat, tok]   (fp32r matmuls)
  v       = xhatT^T @ Wv        [tok, feat]   (+ interleaved ones columns)
  scoresT = kT_h^T @ qT_h       [kpos, q]  per (head, 128-token key chunk)
  eT      = exp(scoresT) * mask{0,1}       (no max subtraction; scores are O(1))
  AV      = [v_h | 1]^T @ eT    [65, 256]: rows 0-63 unnormalized out,
                                row 64 = softmax denominator
  attgT   = AV[0:64] * (sigmoid(gate)/denom)  broadcast along partitions
  yT      = W_out^T @ attgT     [dim, tok]
RMS-norm gamma*sqrt(1024), the 1/sqrt(64) attention scale, and gamma for
the gate projection are folded into the weights host-side. All heavy
matmuls run in fp32r (fp32 with 12-bit mantissa rounding, full PE rate);
attention weights/values use bf16.
"""
import numpy as np
import ml_dtypes

import concourse.bass as bass
import concourse.tile as tile
from concourse import bacc, mybir
from concourse.bass_utils import run_bass_kernel_spmd

F32 = mybir.dt.float32
F32R = mybir.dt.float32r
BF16 = mybir.dt.bfloat16
AF = mybir.ActivationFunctionType

P = 128
DIM = 1024
HEADS = 16
DH = 64
WIN = 256
OWN = 1024          # owned tokens per core
HALO = 256
SL = OWN + HALO     # local tokens (1280)
KK = DIM // P       # 8 contraction chunks
FT = HEADS // 2     # 8 feature tiles (2 heads each)
TCH = SL // P       # 10 local token chunks
NB = OWN // WIN     # 4 owned blocks
NCORES = 8

# q-span (in owned-token coords) of each global key chunk g, and width
_G_SPAN = [(0, 256), (0, 256), (0, 512), (0, 512), (256, 512), (256, 512),
           (512, 512), (512, 512), (768, 256), (768, 256)]
# column offset of chunk g's mask inside the [128, 2048] mask tensor
_G_MASK = [1024, 1280, 0, 0, 0, 0, 0, 0, 1536, 1792]
for _g in (3, 5, 7):
    _G_MASK[_g] = 512
# statically-valid column range of each g's eT tile (outside: mask is 0,
# so exp is skipped there and the mask multiply writes the zeros)
_G_VALID = [(0, 128), (0, 256), (0, 384), (128, 384), (0, 384), (128, 384),
            (0, 384), (128, 384), (0, 256), (128, 128)]


def _round_f32r(a):
    u = np.ascontiguousarray(a, dtype=np.float32).view(np.uint32)
    r = ((u.astype(np.uint64) + 0x800) & 0xFFFFF000).astype(np.uint32)
    return r.view(np.float32).reshape(a.shape)


def _band(c):
    """{0,1} validity for key-chunk-position kp vs in-block query ql."""
    kp = np.arange(P)[:, None]
    ql = np.arange(WIN)[None, :]
    diff = 256 + ql - 128 * c - kp
    return ((diff >= 0) & (diff <= WIN)).astype(np.float32)


def _masks(first_segment):
    m_even = np.concatenate([_band(2), _band(0)], axis=1)
    m_odd = np.concatenate([_band(3), _band(1)], axis=1)
    zeros = np.zeros_like(_band(0))
    g0 = zeros if first_segment else _band(0)
    g1 = zeros if first_segment else _band(1)
    m = np.concatenate([m_even, m_odd, g0, g1, _band(2), _band(3)], axis=1)
    return m.astype(ml_dtypes.bfloat16)


def build():
    nc = bacc.Bacc("TRN2", target_bir_lowering=False, debug=False,
                   num_devices=NCORES)
    xT_d = nc.dram_tensor("xT", [DIM, SL], F32, kind="ExternalInput")
    wq_d = nc.dram_tensor("Wq", [DIM, DIM], F32R, kind="ExternalInput")
    wk_d = nc.dram_tensor("Wk", [DIM, DIM], F32R, kind="ExternalInput")
    wv_d = nc.dram_tensor("Wv", [DIM, DIM], F32R, kind="ExternalInput")
    wg_d = nc.dram_tensor("Wg", [DIM, HEADS], F32R, kind="ExternalInput")
    bg_d = nc.dram_tensor("bg", [HEADS], F32, kind="ExternalInput")
    wo_d = nc.dram_tensor("Wo", [DIM, DIM], BF16, kind="ExternalInput")
    mask_d = nc.dram_tensor("mask", [P, 2048], BF16, kind="ExternalInput")
    out_d = nc.dram_tensor("out", [DIM, OWN], F32, kind="ExternalOutput")

    lsegs = [(0, 512), (512, 512), (1024, 256)]   # local-token segments
    osegs = [(0, 512), (512, 512)]                # owned-token segments

    with tile.TileContext(nc) as tc:
        ps = tc.alloc_tile_pool(name="ps", bufs=8, space="PSUM")

        def psum(shape):
            return ps.tile(shape, F32, tag="ps", name="pst")

        def psum_acc(shape):
            return ps.tile(shape, F32, tag="ps", name="pacc")

        const_p = tc.alloc_tile_pool(name="const", bufs=1, side="left")
        mask_sb = const_p.tile([P, 2048], BF16, bufs=1)
        ones_f = const_p.tile([P, 1], F32, bufs=1)
        nc.vector.memset(ones_f[:], 1.0)
        ones_sb = const_p.tile([P, 1], F32R, bufs=1)
        nc.vector.tensor_copy(ones_sb[:], ones_f[:])
        # NOTE: onesr is unused by the compute, but removing it shifts the
        # Tile schedule and measurably regresses HW time (337us -> 396us).
        onesr_f = const_p.tile([1, DH], F32, bufs=1)
        nc.vector.memset(onesr_f[:], 1.0)
        onesr = const_p.tile([1, DH], F32R, bufs=1)
        nc.vector.tensor_copy(onesr[:], onesr_f[:])
        bg_sb = const_p.tile([HEADS, 1], F32, bufs=1)
        eps_sb = const_p.tile([1, 1], F32, bufs=1)
        nc.vector.memset(eps_sb[:], 1e-24)
        sgT = const_p.tile([HEADS, OWN], F32, bufs=1)

        w_p = tc.alloc_tile_pool(name="w", bufs=14, side="right")
        xh_p = tc.alloc_tile_pool(name="xh", bufs=KK, side="right")
        x_p = tc.alloc_tile_pool(name="x", bufs=KK, side="right")
        x2_p = tc.alloc_tile_pool(name="x2", bufs=3, side="right")

        def wload(dram, kk, name):
            wt = w_p.tile([P, DIM], F32R, tag="w", name=name)
            nc.sync.dma_start(wt[:], dram[kk * P:(kk + 1) * P, :])
            return wt

        x_sb = []
        for kk in range(KK):
            xt = x_p.tile([P, SL], F32, tag="xT", name=f"x{kk}")
            nc.sync.dma_start(xt[:], xT_d[kk * P:(kk + 1) * P, :])
            x_sb.append(xt)
        wq_sb = [wload(wq_d, kk, f"wq{kk}") for kk in range(KK)]
        nc.sync.dma_start(mask_sb[:], mask_d[:])
        nc.sync.dma_start(bg_sb[:], bg_d[:])

        # HAM warm-up: const-fed dummy matmuls keep the PE busy while the
        # xT DMA lands, so projections start at 2.4GHz instead of 1.2GHz.
        dmy_f = const_p.tile([1, 512], F32, bufs=1)
        nc.vector.memset(dmy_f[:], 1.0)
        dmy_r = const_p.tile([1, 512], F32R, bufs=1)
        nc.vector.tensor_copy(dmy_r[:], dmy_f[:])
        warm_ps = psum([DH, 512])
        for j in range(16):
            nc.tensor.matmul(warm_ps[:], onesr[:], dmy_r[:],
                             start=(j == 0), stop=(j == 15))

        # ---- norm: rs = 1/sqrt(sum_d x^2) ------------------------------
        rs_row = x2_p.tile([1, SL], F32, bufs=1)
        rsb = x2_p.tile([P, SL], F32, bufs=1)
        ssq_ps = [psum([1, w]) for _, w in lsegs]
        for kk in range(KK):
            x2 = x2_p.tile([P, SL], F32R, tag="x2", name=f"x2_{kk}")
            nc.scalar.activation(x2[:], x_sb[kk][:], AF.Square)
            for si, (s0, w) in enumerate(lsegs):
                nc.tensor.matmul(ssq_ps[si][:], ones_sb[:], x2[:, s0:s0 + w],
                                 start=(kk == 0), stop=(kk == KK - 1))
        for si, (s0, w) in enumerate(lsegs):
            nrm = x2_p.tile([1, 512], F32, tag="nrm", name=f"nrm{si}")
            nc.scalar.activation(nrm[:1, :w], ssq_ps[si][:], AF.Sqrt,
                                 bias=eps_sb[:])
            nc.vector.reciprocal_approx_fast(rs_row[:, s0:s0 + w],
                                             nrm[:1, :w])
        nc.gpsimd.partition_broadcast(rsb[:], rs_row[:])

        xh_sb = []
        for kk in range(KK):
            xh = xh_p.tile([P, SL], F32R, tag="xh", name=f"xh{kk}")
            nc.vector.tensor_mul(xh[:], x_sb[kk][:], rsb[:])
            xh_sb.append(xh)
        x2_p.release()
        x_p.release()

        # ---- projections ----------------------------------------------
        q_p = tc.alloc_tile_pool(name="q", bufs=FT, side="left")
        k_p = tc.alloc_tile_pool(name="k", bufs=FT, side="left")
        v_p = tc.alloc_tile_pool(name="v", bufs=TCH, side="left")
        wg_p = tc.alloc_tile_pool(name="wg", bufs=KK, side="right")

        # Q pass (owned tokens only)
        qT = [q_p.tile([P, OWN], BF16, tag="qT", name=f"qT{ft}")
              for ft in range(FT)]
        for ft in range(FT):
            for s0, w in osegs:
                acc = psum([P, w])
                for kk in range(KK):
                    nc.tensor.matmul(
                        acc[:], wq_sb[kk][:, ft * P:(ft + 1) * P],
                        xh_sb[kk][:, HALO + s0:HALO + s0 + w],
                        start=(kk == 0), stop=(kk == KK - 1))
                nc.scalar.copy(qT[ft][:, s0:s0 + w], acc[:])

        # K pass (all local tokens)
        wk_sb = [wload(wk_d, kk, f"wk{kk}") for kk in range(KK)]
        kT = [k_p.tile([P, SL], BF16, tag="kT", name=f"kT{ft}")
              for ft in range(FT)]
        for ft in range(FT):
            for s0, w in lsegs:
                acc = psum([P, w])
                for kk in range(KK):
                    nc.tensor.matmul(
                        acc[:], wk_sb[kk][:, ft * P:(ft + 1) * P],
                        xh_sb[kk][:, s0:s0 + w],
                        start=(kk == 0), stop=(kk == KK - 1))
                nc.scalar.copy(kT[ft][:, s0:s0 + w], acc[:])

        # V pass -> token-major with interleaved ones columns
        wv_sb = [wload(wv_d, kk, f"wv{kk}") for kk in range(KK)]
        v_sb = []
        for g in range(TCH):
            vt = v_p.tile([P, HEADS * (DH + 1)], BF16, tag="v", name=f"v{g}")
            v3 = vt.rearrange("p (h e) -> p h e", e=DH + 1)
            nc.vector.memset(v3[:, :, DH:DH + 1], 1.0)
            for fh in range(2):
                acc = psum([P, 512])
                for kk in range(KK):
                    nc.tensor.matmul(
                        acc[:], xh_sb[kk][:, g * P:(g + 1) * P],
                        wv_sb[kk][:, fh * 512:(fh + 1) * 512],
                        start=(kk == 0), stop=(kk == KK - 1))
                nc.vector.tensor_copy(v3[:, 8 * fh:8 * (fh + 1), 0:DH], acc[:])
            v_sb.append(v3)

        # gates -> sigmoid(x @ Wg + bg), head-major [16, 1024]
        wg_sb = []
        for kk in range(KK):
            wgt = wg_p.tile([P, HEADS], F32R, tag="wg", name=f"wgk{kk}")
            nc.sync.dma_start(wgt[:], wg_d[kk * P:(kk + 1) * P, :])
            wg_sb.append(wgt)
        for s0, w in osegs:
            acc = psum([HEADS, w])
            for kk in range(KK):
                nc.tensor.matmul(acc[:], wg_sb[kk][:],
                                 xh_sb[kk][:, HALO + s0:HALO + s0 + w],
                                 start=(kk == 0), stop=(kk == KK - 1))
            nc.scalar.activation(sgT[:, s0:s0 + w], acc[:], AF.Sigmoid,
                                 bias=bg_sb[:])
        wg_p.release()
        xh_p.release()
        w_p.release()

        # ---- attention --------------------------------------------------
        ag_p = tc.alloc_tile_pool(name="ag", bufs=FT, side="right")
        wo_p = tc.alloc_tile_pool(name="wo", bufs=KK, side="right")
        e_p = tc.alloc_tile_pool(name="e", bufs=32, side="right")
        av_p = tc.alloc_tile_pool(name="av", bufs=2, side="right")
        wo_sb = []
        for t in range(KK):
            wot = wo_p.tile([P, DIM], BF16, tag="wo", name=f"wo{t}")
            nc.sync.dma_start(wot[:], wo_d[t * P:(t + 1) * P, :])
            wo_sb.append(wot)
        agT = [ag_p.tile([P, OWN], BF16, tag="agT", name=f"agT{ft}")
               for ft in range(FT)]
        def emit_scores(ft, h2):
            h = 2 * ft + h2
            hp = h2 * DH
            # gate row for head h staged at partition 0 (HW
            # partition_broadcast always reads physical partition 0);
            # DMA is exempt from the engine partition-alignment rules
            sg0 = av_p.tile([1, OWN], F32, tag="sg0", name=f"sg0_{h}",
                            bufs=4)
            nc.sync.dma_start(sg0[:], sgT[h:h + 1, :])
            eT = [None] * TCH
            for g in (2, 3, 0, 1, 6, 7, 4, 5, 8, 9):
                qs, w = _G_SPAN[g]
                v0, vw = _G_VALID[g]
                sc = psum([P, vw])
                nc.tensor.matmul(
                    sc[:], kT[ft][hp:hp + DH, g * P:(g + 1) * P],
                    qT[ft][hp:hp + DH, qs + v0:qs + v0 + vw],
                    start=True, stop=True)
                e = e_p.tile([P, 512], BF16, tag="eT", name=f"e{g}")
                nc.scalar.activation(e[:, v0:v0 + vw], sc[:], AF.Exp)
                mc = _G_MASK[g]
                nc.vector.tensor_mul(e[:, v0:v0 + vw], e[:, v0:v0 + vw],
                                     mask_sb[:, mc + v0:mc + v0 + vw])
                if v0 > 0:
                    nc.vector.memset(e[:, 0:v0], 0.0)
                if v0 + vw < w:
                    nc.vector.memset(e[:, v0 + vw:w], 0.0)
                eT[g] = e
            return sg0, eT

        def emit_av(ft, h2, sg0, eT):
            h = 2 * ft + h2
            hp = h2 * DH
            for i in range(2):
                # block pair (2i, 2i+1): one [65, 512] accumulation.
                # full-width matmuls first so start=True overwrites the
                # whole region before partial-width accumulates land.
                acc = psum_acc([DH + 1, 2 * WIN])
                base = 4 * i
                parts = [(base + 2, 0, 0, 512), (base + 3, 0, 0, 512),
                         (base + 0, 2 * i * WIN - _G_SPAN[base][0], 0, WIN),
                         (base + 1, 2 * i * WIN - _G_SPAN[base + 1][0],
                          0, WIN),
                         (base + 4,
                          (2 * i + 1) * WIN - _G_SPAN[base + 4][0],
                          WIN, WIN),
                         (base + 5,
                          (2 * i + 1) * WIN - _G_SPAN[base + 5][0],
                          WIN, WIN)]
                for j, (g, sect, p0, pw) in enumerate(parts):
                    nc.tensor.matmul(
                        acc[:, p0:p0 + pw], v_sb[g][:, h, :],
                        eT[g][:, sect:sect + pw],
                        start=(j == 0), stop=(j == len(parts) - 1),
                        skip_group_check=True)
                # scale = sigmoid(gate)/denominator; broadcast along
                # partitions via a rank-1 (ones x crow) matmul on PE
                srow = av_p.tile([1, 2 * WIN], F32, tag="srow",
                                 name=f"sr{i}")
                nc.vector.tensor_copy(srow[:], acc[DH:DH + 1, :])
                sinv = av_p.tile([1, 2 * WIN], F32, tag="sinv",
                                 name=f"si{i}")
                nc.vector.reciprocal_approx_fast(sinv[:], srow[:])
                crow = av_p.tile([1, 2 * WIN], F32, tag="crow",
                                 name=f"cr{i}")
                nc.vector.tensor_mul(
                    crow[:], sinv[:],
                    sg0[:, 2 * i * WIN:2 * (i + 1) * WIN])
                cb = av_p.tile([DH, 2 * WIN], F32, tag="cb",
                               name=f"cb{i}")
                nc.gpsimd.partition_broadcast(cb[:], crow[:])
                nc.vector.tensor_mul(
                    agT[ft][hp:hp + DH, 2 * i * WIN:2 * (i + 1) * WIN],
                    acc[0:DH, :], cb[:])


        from collections import deque
        pend = deque()
        for ft in range(FT):
            for h2 in range(2):
                pend.append((ft, h2, *emit_scores(ft, h2)))
                if len(pend) > 1:
                    emit_av(*pend.popleft())
        while pend:
            emit_av(*pend.popleft())

        # ---- output projection -----------------------------------------
        av_p.release()
        e_p.release()
        v_p.release()
        k_p.release()
        q_p.release()
        y_p = tc.alloc_tile_pool(name="y", bufs=3, side="right")
        for dt in range(KK):
            yt = y_p.tile([P, OWN], F32, tag="yt", name=f"yt{dt}")
            for s0, w in osegs:
                acc = psum([P, w])
                for t in range(KK):
                    nc.tensor.matmul(acc[:], wo_sb[t][:, dt * P:(dt + 1) * P],
                                     agT[t][:, s0:s0 + w],
                                     start=(t == 0), stop=(t == KK - 1))
                nc.scalar.copy(yt[:, s0:s0 + w], acc[:])
            nc.sync.dma_start(out_d[dt * P:(dt + 1) * P, :], yt[:])
        y_p.release()
        wo_p.release()
        ag_p.release()
        const_p.release()
        ps.release()

    nc.compile()
    return nc


def make_in_maps(x, gamma, W_qkv, W_gates, b_gates, W_out):
    b, S, dim = x.shape
    assert (b, S, dim) == (2, 4096, DIM)
    g32 = (gamma * (dim ** 0.5)).astype(np.float32)
    wqkv = W_qkv * g32[:, None]
    wq = _round_f32r(wqkv[:, :DIM] * (DH ** -0.5))
    wk = _round_f32r(wqkv[:, DIM:2 * DIM])
    wv = _round_f32r(wqkv[:, 2 * DIM:3 * DIM])
    wg = _round_f32r(W_gates * g32[:, None])
    wo = np.asarray(W_out, np.float32).astype(ml_dtypes.bfloat16)
    bg = np.ascontiguousarray(b_gates, dtype=np.float32)
    m_first = _masks(True)
    m_rest = _masks(False)

    in_maps = []
    for c in range(NCORES):
        bb, seg = c // 4, c % 4
        own = x[bb, seg * OWN:(seg + 1) * OWN]
        halo = x[bb, seg * OWN - HALO: seg * OWN] if seg else x[bb, :HALO]
        xT = np.ascontiguousarray(
            np.concatenate([halo, own], axis=0).T, dtype=np.float32)
        in_maps.append({
            "xT": xT, "Wq": wq, "Wk": wk, "Wv": wv, "Wg": wg, "bg": bg,
            "Wo": wo, "mask": m_first if seg == 0 else m_rest,
        })
    return in_maps


_NC_CACHE = []


def kernel(x, gamma, W_qkv, W_gates, b_gates, W_out):
    x = np.asarray(x, dtype=np.float32)
    in_maps = make_in_maps(
        x, np.asarray(gamma, np.float32), np.asarray(W_qkv, np.float32),
        np.asarray(W_gates, np.float32), np.asarray(b_gates, np.float32),
        np.asarray(W_out, np.float32))
    if not _NC_CACHE:
        _NC_CACHE.append(build())
    nc = _NC_CACHE[0]
    res = run_bass_kernel_spmd(nc, in_maps, core_ids=list(range(NCORES)))
    y = np.empty((2, 4096, DIM), dtype=np.float32)
    for c in range(NCORES):
        bb, seg = c // 4, c % 4
        y[bb, seg * OWN:(seg + 1) * OWN] = res.results[c]["out"].T
    return y



# revision 7
# speedup vs baseline: 1.2545x; 1.2545x over previous
"""Sliding-window gated attention on 8 TRN2 NeuronCores — v2.

Sharding: data/sequence parallel, no collectives. 2 batches x 4096 tokens
-> 8 shards of 1024 owned tokens (core c: batch c//4, segment c%4) plus a
256-token left halo of x; the attention mask zeroes the (dummy) halo for
segment-0 cores.

v2 changes vs the 321us baseline (engine-profile driven):
  * bf16 everywhere instead of fp32r; Q/K projections in fp8e4m3 with
    DoubleRow perf mode (2 contraction chunks per matmul).
  * Merged pipeline: per feature-tile ft, Q/K projection of ft+1 overlaps
    the attention (exp/mask/AV) of ft, keeping the PE warm (HAM K=8/8).
  * Softmax-denominator/gate scaling reworked: per-head [1,512] DVE row
    ops + gpsimd partition_broadcast (70+40us) replaced by a batched
    [16,1024] denominator tile, one reciprocal + one multiply per 2-ft
    group, and a rank-16 PE matmul broadcast (Eall selector) for the
    per-column scale.
  * eT tiles persistent + pre-zeroed once; mask-mul writes only the
    statically-valid span (no per-head memsets).
"""
import numpy as np
import ml_dtypes

import concourse.bass as bass
import concourse.tile as tile
from concourse import bacc, mybir
from concourse.bass_utils import run_bass_kernel_spmd

F32 = mybir.dt.float32
BF16 = mybir.dt.bfloat16
FP8 = mybir.dt.float8e4
AF = mybir.ActivationFunctionType
DR = mybir.MatmulPerfMode.DoubleRow

P = 128
DIM = 1024
HEADS = 16
DH = 64
WIN = 256
OWN = 1024          # owned tokens per core
HALO = 256
SL = OWN + HALO     # local tokens (1280)
KK = DIM // P       # 8 contraction chunks
NPAIR = KK // 2     # 4 fp8 DoubleRow pairs
FT = HEADS // 2     # 8 feature tiles (2 heads each)
TCH = SL // P       # 10 local token chunks
NCORES = 8

USE_FP8 = True      # Q/K projections in fp8e4m3 + DoubleRow
S1 = 128.0          # xhat fp8 scale
S2Q = 128.0         # Wq fp8 scale (applied after the 1/8 attn scale)
S2K = 16.0          # Wk fp8 scale
DESC_Q = 1.0 / (S1 * S2Q)
DESC_K = 1.0 / (S1 * S2K)

# q-span (in owned-token coords) of each global key chunk g, and width
_G_SPAN = [(0, 256), (0, 256), (0, 512), (0, 512), (256, 512), (256, 512),
           (512, 512), (512, 512), (768, 256), (768, 256)]
# column offset of chunk g's mask inside the [128, 2048] mask tensor
_G_MASK = [1024, 1280, 0, 512, 0, 512, 0, 512, 1536, 1792]
# statically-valid column range of each g's eT tile (outside: mask is 0,
# so exp is skipped there; tiles are pre-zeroed once)
_G_VALID = [(0, 128), (0, 256), (0, 384), (128, 384), (0, 384), (128, 384),
            (0, 384), (128, 384), (0, 256), (128, 128)]
_G_ORDER = (2, 3, 0, 1, 6, 7, 4, 5, 8, 9)


def _band(c):
    """{0,1} validity for key-chunk-position kp vs in-block query ql."""
    kp = np.arange(P)[:, None]
    ql = np.arange(WIN)[None, :]
    diff = 256 + ql - 128 * c - kp
    return ((diff >= 0) & (diff <= WIN)).astype(np.float32)


def _masks(first_segment):
    m_even = np.concatenate([_band(2), _band(0)], axis=1)
    m_odd = np.concatenate([_band(3), _band(1)], axis=1)
    zeros = np.zeros_like(_band(0))
    g0 = zeros if first_segment else _band(0)
    g1 = zeros if first_segment else _band(1)
    m = np.concatenate([m_even, m_odd, g0, g1, _band(2), _band(3)], axis=1)
    return m.astype(ml_dtypes.bfloat16)


def _eall():
    """[16, 8*128] bf16: per-ft rank-16 selector for the cb broadcast."""
    e = np.zeros((HEADS, FT * P), dtype=np.float32)
    for ft in range(FT):
        e[2 * ft, ft * P:ft * P + DH] = 1.0
        e[2 * ft + 1, ft * P + DH:(ft + 1) * P] = 1.0
    return e.astype(ml_dtypes.bfloat16)


def build():
    nc = bacc.Bacc("TRN2", target_bir_lowering=False, debug=False,
                   num_devices=NCORES)
    xT_d = nc.dram_tensor("xT", [DIM, SL], BF16, kind="ExternalInput")
    if USE_FP8:
        wq_d = nc.dram_tensor("Wq", [NPAIR, P, 2, DIM], FP8,
                              kind="ExternalInput")
        wk_d = nc.dram_tensor("Wk", [NPAIR, P, 2, DIM], FP8,
                              kind="ExternalInput")
    else:
        wq_d = nc.dram_tensor("Wq", [DIM, DIM], BF16, kind="ExternalInput")
        wk_d = nc.dram_tensor("Wk", [DIM, DIM], BF16, kind="ExternalInput")
    wv_d = nc.dram_tensor("Wv", [DIM, DIM], BF16, kind="ExternalInput")
    wg_d = nc.dram_tensor("Wg", [DIM, HEADS], BF16, kind="ExternalInput")
    bg_d = nc.dram_tensor("bg", [HEADS], F32, kind="ExternalInput")
    wo_d = nc.dram_tensor("Wo", [DIM, DIM], BF16, kind="ExternalInput")
    mask_d = nc.dram_tensor("mask", [P, 2048], BF16, kind="ExternalInput")
    eall_d = nc.dram_tensor("eall", [HEADS, FT * P], BF16,
                            kind="ExternalInput")
    out_d = nc.dram_tensor("out", [DIM, OWN], F32, kind="ExternalOutput")

    lsegs = [(0, 512), (512, 512), (1024, 256)]   # local-token segments
    osegs = [(0, 512), (512, 512)]                # owned-token segments

    with tile.TileContext(nc) as tc:
        # ---- PSUM pools: 2 + 4 + 2 = 8 banks ---------------------------
        ps_proj = tc.alloc_tile_pool(name="psp", bufs=2, space="PSUM")
        ps_sc = tc.alloc_tile_pool(name="pss", bufs=4, space="PSUM")
        ps_av = tc.alloc_tile_pool(name="psa", bufs=2, space="PSUM")

        def proj_ps(shape):
            return ps_proj.tile(shape, F32, tag="proj", name="prps")

        def sc_ps(shape):
            return ps_sc.tile(shape, F32, tag="sc", name="scps")

        def av_ps(shape):
            return ps_av.tile(shape, F32, tag="av", name="avps")

        # ---- constants / persistent small tiles ------------------------
        const_p = tc.alloc_tile_pool(name="const", bufs=1, side="left")
        mask_sb = const_p.tile([P, 2048], BF16, bufs=1)
        eall_sb = const_p.tile([HEADS, FT * P], BF16, bufs=1)
        ones_f = const_p.tile([P, 1], F32, bufs=1)
        nc.vector.memset(ones_f[:], 1.0)
        ones_b = const_p.tile([P, 1], BF16, bufs=1)
        nc.vector.tensor_copy(ones_b[:], ones_f[:])
        bg_sb = const_p.tile([HEADS, 1], F32, bufs=1)
        eps_sb = const_p.tile([1, 1], F32, bufs=1)
        nc.vector.memset(eps_sb[:], 1e-24)
        sgT = const_p.tile([HEADS, OWN], F32, bufs=1)
        den16 = const_p.tile([HEADS, OWN], F32, bufs=1)
        c16 = const_p.tile([HEADS, OWN], BF16, bufs=1)
        nc.vector.memset(c16[:], 0.0)

        nc.sync.dma_start(mask_sb[:], mask_d[:])
        nc.sync.dma_start(eall_sb[:], eall_d[:])
        nc.sync.dma_start(bg_sb[:], bg_d[:])

        # ---- right-side pool stack (bottom -> top = long -> short lived)
        w_p = tc.alloc_tile_pool(name="w", bufs=1, side="right")
        e_p = tc.alloc_tile_pool(name="e", bufs=1, side="right")
        ag_p = tc.alloc_tile_pool(name="ag", bufs=1, side="right")
        qk_p = tc.alloc_tile_pool(name="qk", bufs=2, side="right")
        nrm_p = tc.alloc_tile_pool(name="nrm2", bufs=2, side="right")
        y_p = tc.alloc_tile_pool(name="y", bufs=2, side="right")
        xh8_p = tc.alloc_tile_pool(name="xh8", bufs=NPAIR, side="right")
        xh_p = tc.alloc_tile_pool(name="xh", bufs=KK, side="right")
        x_p = tc.alloc_tile_pool(name="x", bufs=KK, side="right")
        x2_p = tc.alloc_tile_pool(name="x2", bufs=2, side="right")

        x_sb = []
        for kk in range(KK):
            xt = x_p.tile([P, SL], BF16, tag="xT", name=f"x{kk}")
            nc.sync.dma_start(xt[:], xT_d[kk * P:(kk + 1) * P, :])
            x_sb.append(xt)
        wg_sb = []
        for kk in range(KK):
            wgt = w_p.tile([P, HEADS], BF16, tag=f"wg{kk}", name=f"wg{kk}",
                           bufs=1)
            nc.sync.dma_start(wgt[:], wg_d[kk * P:(kk + 1) * P, :])
            wg_sb.append(wgt)
        wv_sb = []
        for kk in range(KK):
            wvt = w_p.tile([P, DIM], BF16, tag=f"wv{kk}", name=f"wv{kk}",
                           bufs=1)
            nc.sync.dma_start(wvt[:], wv_d[kk * P:(kk + 1) * P, :])
            wv_sb.append(wvt)
        if USE_FP8:
            wq_sb, wk_sb = [], []
            for pr in range(NPAIR):
                wqt = w_p.tile([P, 2, DIM], FP8, tag=f"wq{pr}",
                               name=f"wq{pr}", bufs=1)
                nc.sync.dma_start(wqt[:], wq_d[pr])
                wq_sb.append(wqt)
            for pr in range(NPAIR):
                wkt = w_p.tile([P, 2, DIM], FP8, tag=f"wk{pr}",
                               name=f"wk{pr}", bufs=1)
                nc.sync.dma_start(wkt[:], wk_d[pr])
                wk_sb.append(wkt)
        else:
            wq_sb, wk_sb = [], []
            for kk in range(KK):
                wqt = w_p.tile([P, DIM], BF16, tag=f"wq{kk}",
                               name=f"wq{kk}", bufs=1)
                nc.sync.dma_start(wqt[:], wq_d[kk * P:(kk + 1) * P, :])
                wq_sb.append(wqt)
            for kk in range(KK):
                wkt = w_p.tile([P, DIM], BF16, tag=f"wk{kk}",
                               name=f"wk{kk}", bufs=1)
                nc.sync.dma_start(wkt[:], wk_d[kk * P:(kk + 1) * P, :])
                wk_sb.append(wkt)

        # HAM warm-up: const-fed dummy matmuls keep the PE busy while the
        # xT DMA lands (PE ramps 1.2 -> 2.4 GHz after ~3.4us busy).
        dmy_f = const_p.tile([1, 512], F32, bufs=1)
        nc.vector.memset(dmy_f[:], 1.0)
        dmy_b = const_p.tile([1, 512], BF16, bufs=1)
        nc.vector.tensor_copy(dmy_b[:], dmy_f[:])
        onesr_b = const_p.tile([1, DH], BF16, bufs=1)
        nc.vector.memset(onesr_b[:], 1.0)
        warm_ps = sc_ps([DH, 512])
        for j in range(16):
            nc.tensor.matmul(warm_ps[:], onesr_b[:], dmy_b[:],
                             start=(j == 0), stop=(j == 15))

        # ---- norm: rs = 1/sqrt(sum_d x^2) ------------------------------
        rs32 = const_p.tile([1, SL], F32, bufs=1)
        rs_row = const_p.tile([1, SL], BF16, bufs=1)
        rsb = const_p.tile([P, SL], BF16, bufs=1)
        ssq_ps = [sc_ps([1, w]) for _, w in lsegs]
        for kk in range(KK):
            x2 = x2_p.tile([P, SL], BF16, tag="x2", name=f"x2_{kk}")
            nc.scalar.activation(x2[:], x_sb[kk][:], AF.Square)
            for si, (s0, w) in enumerate(lsegs):
                nc.tensor.matmul(ssq_ps[si][:], ones_b[:], x2[:, s0:s0 + w],
                                 start=(kk == 0), stop=(kk == KK - 1))
        for si, (s0, w) in enumerate(lsegs):
            nrm = x2_p.tile([1, 512], F32, tag="nrm", name=f"nrm{si}")
            nc.scalar.activation(nrm[:1, :w], ssq_ps[si][:], AF.Sqrt,
                                 bias=eps_sb[:])
            nc.vector.reciprocal_approx_fast(rs32[:, s0:s0 + w],
                                             nrm[:1, :w])
        nc.vector.tensor_copy(rs_row[:], rs32[:])
        nc.gpsimd.partition_broadcast(rsb[:], rs_row[:])

        # ---- gates: sgT = sigmoid((x @ Wg) * rs + bg), [16, 1024] ------
        for s0, w in osegs:
            gacc = sc_ps([HEADS, w])
            for kk in range(KK):
                nc.tensor.matmul(gacc[:], wg_sb[kk][:],
                                 x_sb[kk][:, HALO + s0:HALO + s0 + w],
                                 start=(kk == 0), stop=(kk == KK - 1))
            gmul = x2_p.tile([HEADS, 512], F32, tag="gmul", name=f"gm{s0}")
            nc.vector.tensor_mul(gmul[:, :w], gacc[:],
                                 rsb[0:HEADS, HALO + s0:HALO + s0 + w])
            nc.scalar.activation(sgT[:, s0:s0 + w], gmul[:, :w], AF.Sigmoid,
                                 bias=bg_sb[:])

        # ---- xhat (bf16) + fp8 copy ------------------------------------
        xh_sb = []
        for kk in range(KK):
            xh = xh_p.tile([P, SL], BF16, tag="xh", name=f"xh{kk}")
            nc.vector.tensor_mul(xh[:], x_sb[kk][:], rsb[:])
            xh_sb.append(xh)
        if USE_FP8:
            xh8_sb = []
            for pr in range(NPAIR):
                x8 = xh8_p.tile([P, 2, SL], FP8, tag="xh8", name=f"xh8_{pr}")
                for j in range(2):
                    nc.vector.tensor_scalar_mul(x8[:, j, :],
                                                xh_sb[2 * pr + j][:], S1)
                xh8_sb.append(x8)
        x2_p.release()
        x_p.release()

        # ---- V pass -> token-major with interleaved ones columns -------
        v_p = tc.alloc_tile_pool(name="v", bufs=TCH, side="left")
        v_sb = []
        for g in range(TCH):
            vt = v_p.tile([P, HEADS * (DH + 1)], BF16, tag="v", name=f"v{g}")
            v3 = vt.rearrange("p (h e) -> p h e", e=DH + 1)
            nc.vector.memset(v3[:, :, DH:DH + 1], 1.0)
            for fh in range(2):
                acc = proj_ps([P, 512])
                for kk in range(KK):
                    nc.tensor.matmul(
                        acc[:], xh_sb[kk][:, g * P:(g + 1) * P],
                        wv_sb[kk][:, fh * 512:(fh + 1) * 512],
                        start=(kk == 0), stop=(kk == KK - 1))
                nc.vector.tensor_copy(v3[:, 8 * fh:8 * (fh + 1), 0:DH],
                                      acc[:])
            v_sb.append(v3)
        if not USE_FP8:
            xh_keep = xh_sb  # bf16 path: Q/K read xh directly
        else:
            xh_p.release()

        # wo DMA + SBUF slot reuses the freed x/xh space
        wo_p = tc.alloc_tile_pool(name="wo", bufs=1, side="right")
        wo_sb = []
        for t in range(KK):
            wot = wo_p.tile([P, DIM], BF16, tag=f"wo{t}", name=f"wo{t}",
                            bufs=1)
            nc.sync.dma_start(wot[:], wo_d[t * P:(t + 1) * P, :])
            wo_sb.append(wot)

        # ---- persistent attention buffers ------------------------------
        # 2 sets (one per h2); exp of ft+1 orders behind AV of ft anyway
        eT = [[e_p.tile([P, 512], BF16, tag=f"e{s}_{g}", name=f"e{s}_{g}",
                        bufs=1)
               for g in range(TCH)] for s in range(2)]
        for s in range(2):
            for g in range(TCH):
                nc.vector.memset(eT[s][g][:], 0.0)
        agTu = [ag_p.tile([P, OWN], BF16, tag=f"agu{ft}", name=f"agu{ft}",
                          bufs=1) for ft in range(FT)]
        agT = agTu  # final scale is applied in place

        # ---- per-ft pipeline -------------------------------------------
        def emit_qk(ft):
            qT = qk_p.tile([P, OWN], BF16, tag="qT", name=f"qT{ft}", bufs=2)
            kT = qk_p.tile([P, SL], BF16, tag="kT", name=f"kT{ft}", bufs=2)
            if USE_FP8:
                for s0, w in osegs:
                    acc = proj_ps([P, w])
                    for pr in range(NPAIR):
                        nc.tensor.matmul(
                            acc[:], wq_sb[pr][:, :, ft * P:(ft + 1) * P],
                            xh8_sb[pr][:, :, HALO + s0:HALO + s0 + w],
                            start=(pr == 0), stop=(pr == NPAIR - 1),
                            perf_mode=DR)
                    nc.scalar.activation(qT[:, s0:s0 + w], acc[:], AF.Copy,
                                         scale=DESC_Q)
                for s0, w in lsegs:
                    acc = proj_ps([P, w])
                    for pr in range(NPAIR):
                        nc.tensor.matmul(
                            acc[:], wk_sb[pr][:, :, ft * P:(ft + 1) * P],
                            xh8_sb[pr][:, :, s0:s0 + w],
                            start=(pr == 0), stop=(pr == NPAIR - 1),
                            perf_mode=DR)
                    nc.scalar.activation(kT[:, s0:s0 + w], acc[:], AF.Copy,
                                         scale=DESC_K)
            else:
                for s0, w in osegs:
                    acc = proj_ps([P, w])
                    for kk in range(KK):
                        nc.tensor.matmul(
                            acc[:], wq_sb[kk][:, ft * P:(ft + 1) * P],
                            xh_sb[kk][:, HALO + s0:HALO + s0 + w],
                            start=(kk == 0), stop=(kk == KK - 1))
                    nc.scalar.copy(qT[:, s0:s0 + w], acc[:])
                for s0, w in lsegs:
                    acc = proj_ps([P, w])
                    for kk in range(KK):
                        nc.tensor.matmul(
                            acc[:], wk_sb[kk][:, ft * P:(ft + 1) * P],
                            xh_sb[kk][:, s0:s0 + w],
                            start=(kk == 0), stop=(kk == KK - 1))
                    nc.scalar.copy(kT[:, s0:s0 + w], acc[:])
            return qT, kT

        def emit_scores_pair(ft, qT, kT):
            """Packed scores: both heads interleaved per key chunk so the
            two K=64 matmuls (row groups 0-1 vs 2-3) overlap in the PE."""
            sets = [eT[h2] for h2 in range(2)]
            for g in _G_ORDER:
                qs, w = _G_SPAN[g]
                v0, vw = _G_VALID[g]
                mc = _G_MASK[g]
                for h2 in range(2):
                    hp = h2 * DH
                    sc = sc_ps([P, vw])
                    nc.tensor.matmul(
                        sc[:], kT[hp:hp + DH, g * P:(g + 1) * P],
                        qT[hp:hp + DH, qs + v0:qs + v0 + vw],
                        start=True, stop=True)
                    e = sets[h2][g]
                    nc.scalar.activation(e[:, v0:v0 + vw], sc[:], AF.Exp)
                    nc.vector.tensor_mul(e[:, v0:v0 + vw], e[:, v0:v0 + vw],
                                         mask_sb[:, mc + v0:mc + v0 + vw])
            return sets

        def emit_av(ft, h2, sets):
            h = 2 * ft + h2
            hp = h2 * DH
            es = sets[h2]
            for i in range(2):
                # block pair (2i, 2i+1): one [65, 512] accumulation.
                # full-width matmuls first so start=True overwrites the
                # whole region before partial-width accumulates land.
                acc = av_ps([DH + 1, 2 * WIN])
                base = 4 * i
                parts = [(base + 2, 0, 0, 512), (base + 3, 0, 0, 512),
                         (base + 0, 2 * i * WIN - _G_SPAN[base][0], 0, WIN),
                         (base + 1, 2 * i * WIN - _G_SPAN[base + 1][0],
                          0, WIN),
                         (base + 4,
                          (2 * i + 1) * WIN - _G_SPAN[base + 4][0],
                          WIN, WIN),
                         (base + 5,
                          (2 * i + 1) * WIN - _G_SPAN[base + 5][0],
                          WIN, WIN)]
                for j, (g, sect, p0, pw) in enumerate(parts):
                    nc.tensor.matmul(
                        acc[:, p0:p0 + pw], v_sb[g][:, h, :],
                        es[g][:, sect:sect + pw],
                        start=(j == 0), stop=(j == len(parts) - 1),
                        skip_group_check=True)
                # unnormalized output + denominator row (DMA: engines
                # cannot address partition base h; DMA is exempt)
                nc.vector.tensor_copy(
                    agTu[ft][hp:hp + DH, 2 * i * WIN:2 * (i + 1) * WIN],
                    acc[0:DH, :])
                dst = nrm_p.tile([1, 2 * WIN], F32, tag="dst",
                                 name=f"dst{h}_{i}", bufs=4)
                nc.vector.tensor_copy(dst[:], acc[DH:DH + 1, :])
                nc.sync.dma_start(
                    den16[h:h + 1, 2 * i * WIN:2 * (i + 1) * WIN], dst[:])

        def emit_norm():
            """c16 = sigmoid(gate)/denominator for all 16 heads, then the
            per-ft column-scale via a rank-16 PE broadcast."""
            inv16 = nrm_p.tile([HEADS, OWN], F32, tag="inv", name="inv16",
                               bufs=1)
            nc.vector.reciprocal_approx_fast(inv16[:], den16[:])
            nc.vector.tensor_mul(c16[:], inv16[:], sgT[:])
            for ft in range(FT):
                cb = nrm_p.tile([P, OWN], BF16, tag="cb", name=f"cb{ft}",
                                bufs=2)
                for i2, (s0, w) in enumerate(osegs):
                    cbp = av_ps([P, w])
                    nc.tensor.matmul(cbp[:],
                                     eall_sb[:, ft * P:(ft + 1) * P],
                                     c16[:, s0:s0 + w],
                                     start=True, stop=True)
                    nc.vector.tensor_copy(cb[:, s0:s0 + w], cbp[:])
                nc.vector.tensor_mul(agTu[ft][:], agTu[ft][:], cb[:])

        qT, kT = emit_qk(0)
        for ft in range(FT):
            sets = emit_scores_pair(ft, qT, kT)
            if ft + 1 < FT:
                qT, kT = emit_qk(ft + 1)
            emit_av(ft, 0, sets)
            emit_av(ft, 1, sets)

        # keep the PE busy across the norm tail (recip+mul on DVE)
        warm2 = sc_ps([DH, 512])
        for j in range(10):
            nc.tensor.matmul(warm2[:], onesr_b[:], dmy_b[:],
                             start=(j == 0), stop=(j == 9))
        emit_norm()

        # ---- output projection -----------------------------------------
        for dt in range(KK):
            for s0, w in osegs:
                yt = y_p.tile([P, 512], F32, tag="yt", name=f"yt{dt}_{s0}")
                acc = proj_ps([P, w])
                for t in range(KK):
                    nc.tensor.matmul(acc[:], wo_sb[t][:, dt * P:(dt + 1) * P],
                                     agT[t][:, s0:s0 + w],
                                     start=(t == 0), stop=(t == KK - 1))
                nc.scalar.copy(yt[:, :w], acc[:])
                nc.sync.dma_start(out_d[dt * P:(dt + 1) * P, s0:s0 + w],
                                  yt[:, :w])

        wo_p.release()
        if not USE_FP8:
            xh_p.release()
        xh8_p.release()
        y_p.release()
        nrm_p.release()
        qk_p.release()
        ag_p.release()
        e_p.release()
        w_p.release()
        v_p.release()
        const_p.release()
        ps_av.release()
        ps_sc.release()
        ps_proj.release()

    nc.compile()
    return nc


def make_in_maps(x, gamma, W_qkv, W_gates, b_gates, W_out):
    b, S, dim = x.shape
    assert (b, S, dim) == (2, 4096, DIM)
    BF = ml_dtypes.bfloat16
    F8NP = ml_dtypes.float8_e4m3fn
    g32 = (np.asarray(gamma, np.float64) * (dim ** 0.5))
    wqkv = np.asarray(W_qkv, np.float64) * g32[:, None]
    wq = wqkv[:, :DIM] * (DH ** -0.5)
    wk = wqkv[:, DIM:2 * DIM]
    wv = wqkv[:, 2 * DIM:3 * DIM].astype(np.float32).astype(BF)
    if USE_FP8:
        wq8 = np.asarray(wq * S2Q, np.float32).astype(F8NP)
        wk8 = np.asarray(wk * S2K, np.float32).astype(F8NP)
        # pair-interleave: [4, 128, 2, 1024]
        wq8 = np.ascontiguousarray(
            wq8.reshape(NPAIR, 2, P, DIM).transpose(0, 2, 1, 3))
        wk8 = np.ascontiguousarray(
            wk8.reshape(NPAIR, 2, P, DIM).transpose(0, 2, 1, 3))
    else:
        wq8 = np.asarray(wq, np.float32).astype(BF)
        wk8 = np.asarray(wk, np.float32).astype(BF)
    wg = (np.asarray(W_gates, np.float64) * g32[:, None]).astype(
        np.float32).astype(BF)
    wo = np.asarray(W_out, np.float32).astype(BF)
    bg = np.ascontiguousarray(b_gates, dtype=np.float32)
    eall = _eall()
    m_first = _masks(True)
    m_rest = _masks(False)

    in_maps = []
    for c in range(NCORES):
        bb, seg = c // 4, c % 4
        own = x[bb, seg * OWN:(seg + 1) * OWN]
        halo = x[bb, seg * OWN - HALO: seg * OWN] if seg else x[bb, :HALO]
        xT = np.ascontiguousarray(
            np.concatenate([halo, own], axis=0).T.astype(np.float32)
        ).astype(BF)
        in_maps.append({
            "xT": xT, "Wq": wq8, "Wk": wk8, "Wv": wv, "Wg": wg, "bg": bg,
            "Wo": wo, "eall": eall,
            "mask": m_first if seg == 0 else m_rest,
        })
    return in_maps


_NC_CACHE = []


def kernel(x, gamma, W_qkv, W_gates, b_gates, W_out):
    x = np.asarray(x, dtype=np.float32)
    in_maps = make_in_maps(
        x, np.asarray(gamma, np.float32), np.asarray(W_qkv, np.float32),
        np.asarray(W_gates, np.float32), np.asarray(b_gates, np.float32),
        np.asarray(W_out, np.float32))
    if not _NC_CACHE:
        _NC_CACHE.append(build())
    nc = _NC_CACHE[0]
    res = run_bass_kernel_spmd(nc, in_maps, core_ids=list(range(NCORES)))
    y = np.empty((2, 4096, DIM), dtype=np.float32)
    for c in range(NCORES):
        bb, seg = c // 4, c % 4
        y[bb, seg * OWN:(seg + 1) * OWN] = res.results[c]["out"].T
    return y
